# revision 49
# baseline (speedup 1.0000x reference)
"""Trainium2 Bass kernel for nn_DeterministicAdjacency (gnn_message_passing).

Math (reference):
    hi = z @ W1[:D]            # (K, E)
    hj = z @ W1[D:]            # (K, E)
    h  = silu(hi[:,None,:] + hj[None,:,:] + b1)    # (K, K, E)
    logits = einsum('ije,eo->ij', h, W2) + b2      # (K, K)
    out = softmax(logits, axis=-1)

b2 is dropped: softmax is invariant to a constant shift.

Sharding: rows (i / query dim) split across 8 cores, 256 rows each. Each core
computes its 256 rows of logits against the full z and does local row softmax.

Per-core layout ("layout A", e on partitions):
  - hjbT2 (128p=(s,e), 2048f=j): hj^T + b1, duplicated on both partition
    halves (s = row-parity slot). Computed once, reused for every row pair.
  - hibP (128p=(s,e), 128f=k): bias columns; column k holds
    [hi[2k,:] ; hi[2k+1,:]] so one ScalarE activation instruction computes
    silu for TWO query rows x all 2048 keys x all 64 features:
        h_k[(s,e), j] = Silu(hjbT2[(s,e), j] + hibP[(s,e), k])
    128 activation instructions total = the ACT roofline for this problem.
  - contraction over e via TensorE: stationary stat_kk (128x128) holds W2
    block-diagonally (stat[(s,e), i_loc] = W2[e] iff i_loc == 2*kk+s), so
    each pair's matmul deposits its two logits rows at the right partitions
    of a (128, 512) PSUM accumulator; 64 pairs accumulate into a full
    128-row logits tile. float32r gives the 1 cycle/row PE path.
  - softmax fused on the PSUM accumulators (DVE max, ACT exp with -max bias
    and accum_out row sums, DVE reciprocal + scale), then DMA out.
"""

import numpy as np

import concourse.bass as bass
import concourse.bacc as bacc
import concourse.mybir as mybir
from concourse import tile
from concourse.bass_utils import run_bass_kernel_spmd

K, D, E = 2048, 128, 64
NCORES = 8
R = K // NCORES            # 256 rows per core
NPAIR = 64                 # row pairs per 128-row i-tile
NT = 4                     # 512-wide j tiles
F32 = mybir.dt.float32
F32R = mybir.dt.float32r
F16 = mybir.dt.float16
AF = mybir.ActivationFunctionType
AX = mybir.AxisListType


def build_nc() -> bass.Bass:
    # Bacc (not raw Bass): its finalize() runs generate_event_semaphores(),
    # which splits multi-sem waits — TRN2 instructions hold at most one wait.
    nc = bacc.Bacc(None, target_bir_lowering=False)
    # zT/zcT come in fp16 and pre-transposed (host layout prep): plain
    # contiguous DMAs, d already on partitions for the hi/hj contractions,
    # and fp16 matmuls run 1 cyc/row.
    zT_d = nc.declare_dram_parameter("zT", [D, K], F16, isOutput=False)
    zcT_d = nc.declare_dram_parameter("zcT", [D, R], F16, isOutput=False)
    # w1a2/w1b2 = [W1a | W1a], [W1b | W1b]: one matmul emits both
    # partition-halves of the (s,e)-duplicated layouts directly.
    w1a2 = nc.declare_dram_parameter("w1a2", [D, 128], F16, isOutput=False)
    w1b2 = nc.declare_dram_parameter("w1b2", [D, 128], F16, isOutput=False)
    b1c2 = nc.declare_dram_parameter("b1c2", [128, 1], F32, isOutput=False)
    stat = nc.declare_dram_parameter("stat", [128, NPAIR, 128], F16, isOutput=False)
    out = nc.declare_dram_parameter("out", [R, K], F32, isOutput=True)

    with tile.TileContext(nc) as tc:
        with tc.tile_pool(name="singles", bufs=1) as singles:
            w1a_sb = singles.tile([D, 128], F16)
            w1b_sb = singles.tile([D, 128], F16)
            b1_sb = singles.tile([128, 1], F32)
            stat_sb = singles.tile([128, NPAIR, 128], F16)
            zT = singles.tile([128, K], F16)
            zcT = singles.tile([128, R], F16)
            hjbT2 = singles.tile([128, K], F32)
            hibP = singles.tile([128, 2 * NPAIR], F32)

            # plain contiguous loads; zT first (it gates the hjbT2 chain)
            # and chunked so each pj matmul starts as its slice lands;
            # stat (2 MB) last — needed ~15us in.
            for t in range(NT):
                nc.sync.dma_start(
                    out=zT[:, t * 512 : (t + 1) * 512],
                    in_=zT_d[:, t * 512 : (t + 1) * 512],
                )
            nc.sync.dma_start(out=zcT[:], in_=zcT_d[:])
            nc.sync.dma_start(out=w1a_sb[:], in_=w1a2[:])
            nc.sync.dma_start(out=w1b_sb[:], in_=w1b2[:])
            nc.sync.dma_start(out=b1_sb[:], in_=b1c2[:])
            nc.sync.dma_start(out=stat_sb[:], in_=stat[:])

            # ---- prologue: hi / hj projections ----
            with tc.tile_pool(name="pp", bufs=1, space="PSUM") as pp:
                # hiT (both halves) -> pair-bias columns; lane-aligned copies
                # (even columns land on the s=0 half, odd on s=1).
                ph = pp.tile([128, R], F32, tag="ph")
                nc.tensor.matmul(ph[:], w1a_sb[:], zcT[:], start=True, stop=True)
                phr = ph.rearrange("e (k two) -> e two k", two=2)
                nc.vector.tensor_copy(hibP[0:E, :], phr[0:E, 0, :])
                nc.vector.tensor_copy(hibP[E:128, :], phr[E:128, 1, :])

                for t in range(NT):
                    # hjT + b1, both (s,e) halves at once via [W1b|W1b].
                    pj = pp.tile([128, 512], F32, tag="pj", bufs=2)
                    nc.tensor.matmul(
                        pj[:], w1b_sb[:], zT[:, t * 512 : (t + 1) * 512],
                        start=True, stop=True,
                    )
                    nc.vector.tensor_scalar_add(
                        out=hjbT2[:, t * 512 : (t + 1) * 512],
                        in0=pj[:], scalar1=b1_sb[:],
                    )

            # ---- main loop: silu + e-contraction into PSUM accumulators ----
            with (
                tc.tile_pool(name="accp", bufs=1, space="PSUM") as accp,
                tc.tile_pool(name="hp", bufs=8) as hp,
                tc.tile_pool(name="ep", bufs=1) as ep,
                tc.tile_pool(name="sp", bufs=4) as sp,
            ):
                acc = {
                    (u, t): accp.tile(
                        [128, 512], F32, tag=f"a{u}{t}", name=f"acc{u}{t}"
                    )
                    for u in range(R // 128)
                    for t in range(NT)
                }
                def contract(k, h_ap):
                    """4 matmuls: acc rows 2kk,2kk+1 += W2-block @ silu tile"""
                    u, kk = divmod(k, NPAIR)
                    st = stat_sb[:, kk, :]
                    for t in range(NT):
                        nc.tensor.matmul(
                            acc[(u, t)][:],
                            st,
                            h_ap[:, t * 512 : (t + 1) * 512],
                            start=(kk == 0),
                            stop=(kk == NPAIR - 1),
                        )

                # Warm-up pairs on the per-pair path (no DVE dependency, so
                # silu starts the moment hjbT2/hibP are ready; also covers
                # the window where the stat DMA is still landing).
                WARM = 8
                for k in range(WARM):
                    h = hp.tile([128, K], F16, tag="h")
                    nc.scalar.activation(
                        out=h[:], in_=hjbT2[:], func=AF.Silu,
                        bias=hibP[:, k : k + 1], scale=1.0,
                    )
                    contract(k, h)

                # Steady state: DVE precomputes x = hjbT2 + bias for 4 pairs
                # (2x_2P mode), then ONE 8192-wide ScalarE silu covers all 4 —
                # amortizes the per-instruction SBUF-latency bubble.
                G = 4
                TAIL = 4  # last pairs go per-pair so the final MM+softmax
                # chain after the last silu is short
                for k0 in range(WARM, R // 2 - TAIL, G):
                    xg = hp.tile([128, G, K], F16, tag="xg", bufs=2)
                    hg = hp.tile([128, G, K], F16, tag="hg", bufs=2)
                    for g in range(G):
                        nc.vector.tensor_scalar_add(
                            out=xg[:, g, :], in0=hjbT2[:],
                            scalar1=hibP[:, k0 + g : k0 + g + 1],
                        )
                    nc.scalar.activation(
                        out=hg.rearrange("p g j -> p (g j)"),
                        in_=xg.rearrange("p g j -> p (g j)"),
                        func=AF.Silu,
                    )
                    for g in range(G):
                        contract(k0 + g, hg[:, g, :])

                for k in range(R // 2 - TAIL, R // 2):
                    h = hp.tile([128, K], F16, tag="h")
                    nc.scalar.activation(
                        out=h[:], in_=hjbT2[:], func=AF.Silu,
                        bias=hibP[:, k : k + 1], scale=1.0,
                    )
                    contract(k, h)

                # ---- fused row softmax + store ----
                # logits are O(+-6) here, so exp without max-subtraction is
                # safe in fp32 and drops the serial max chain from the tail.
                for u in range(R // 128):
                    sums = sp.tile([128, NT], F32, tag="sums")
                    tot = sp.tile([128, 1], F32, tag="tot")
                    rec = sp.tile([128, 1], F32, tag="rec")
                    ex = ep.tile([128, K], F32, tag=f"ex{u}")
                    for t in range(NT):
                        nc.scalar.activation(
                            out=ex[:, t * 512 : (t + 1) * 512],
                            in_=acc[(u, t)][:], func=AF.Exp,
                            accum_out=sums[:, t : t + 1],
                        )
                    nc.vector.reduce_sum(out=tot[:], in_=sums[:], axis=AX.X)
                    nc.vector.reciprocal(out=rec[:], in_=tot[:])
                    # chunked normalize+store so the DMA overlaps the scale
                    for c in range(2):
                        sl = slice(c * (K // 2), (c + 1) * (K // 2))
                        nc.vector.tensor_scalar_mul(
                            out=ex[:, sl], in0=ex[:, sl], scalar1=rec[:]
                        )
                        nc.sync.dma_start(
                            out=out[u * 128 : (u + 1) * 128, sl], in_=ex[:, sl]
                        )
    nc.finalize()  # Bacc.compile(): wait splitting, reg alloc, act tables
    return nc


_CACHE: dict = {}


def _get_nc() -> bass.Bass:
    if "nc" not in _CACHE:
        _CACHE["nc"] = build_nc()
    return _CACHE["nc"]


def make_in_maps(z, W1, b1, W2):
    z = np.ascontiguousarray(np.asarray(z, np.float32))
    W1 = np.asarray(W1, np.float32)
    b1 = np.asarray(b1, np.float32)
    W2 = np.asarray(W2, np.float32)

    stat = np.zeros((128, NPAIR, 128), np.float32)
    w2col = W2[:, 0]
    for kk in range(NPAIR):
        for s in range(2):
            stat[s * E : (s + 1) * E, kk, 2 * kk + s] = w2col
    stat = stat.astype(np.float16)
    b1c2 = np.ascontiguousarray(np.tile(b1, 2).reshape(128, 1))
    w1a2 = np.ascontiguousarray(np.tile(W1[:D], (1, 2)).astype(np.float16))
    w1b2 = np.ascontiguousarray(np.tile(W1[D:], (1, 2)).astype(np.float16))
    zT16 = np.ascontiguousarray(z.astype(np.float16).T)  # (D, K)

    in_maps = []
    for c in range(NCORES):
        in_maps.append(
            {
                "zT": zT16,
                "zcT": np.ascontiguousarray(zT16[:, c * R : (c + 1) * R]),
                "w1a2": w1a2,
                "w1b2": w1b2,
                "b1c2": b1c2,
                "stat": stat,
            }
        )
    return in_maps


def run(inputs: dict, trace: bool = False):
    """Run the bass kernel; returns (full_output, BassKernelResults)."""
    nc = _get_nc()
    in_maps = make_in_maps(inputs["z"], inputs["W1"], inputs["b1"], inputs["W2"])
    res = run_bass_kernel_spmd(nc, in_maps, list(range(NCORES)), trace=trace)
    full = np.concatenate([res.results[c]["out"] for c in range(NCORES)], axis=0)
    return full, res


def kernel(**inputs) -> np.ndarray:
    full, _ = run(inputs, trace=False)
    return full


# revision 50
# speedup vs baseline: 1.0080x; 1.0080x over previous
"""Trainium2 Bass kernel for nn_DeterministicAdjacency (gnn_message_passing).

Math (reference):
    hi = z @ W1[:D]            # (K, E)
    hj = z @ W1[D:]            # (K, E)
    h  = silu(hi[:,None,:] + hj[None,:,:] + b1)    # (K, K, E)
    logits = einsum('ije,eo->ij', h, W2) + b2      # (K, K)
    out = softmax(logits, axis=-1)

b2 is dropped: softmax is invariant to a constant shift.

Sharding: rows (i / query dim) split across 8 cores, 256 rows each. Each core
computes its 256 rows of logits against the full z and does local row softmax.

Per-core layout ("layout A", e on partitions):
  - hjbT2 (128p=(s,e), 2048f=j): hj^T + b1, duplicated on both partition
    halves (s = row-parity slot). Computed once, reused for every row pair.
  - hibP (128p=(s,e), 128f=k): bias columns; column k holds
    [hi[2k,:] ; hi[2k+1,:]] so one ScalarE activation instruction computes
    silu for TWO query rows x all 2048 keys x all 64 features:
        h_k[(s,e), j] = Silu(hjbT2[(s,e), j] + hibP[(s,e), k])
    128 activation instructions total = the ACT roofline for this problem.
  - contraction over e via TensorE: stationary stat_kk (128x128) holds W2
    block-diagonally (stat[(s,e), i_loc] = W2[e] iff i_loc == 2*kk+s), so
    each pair's matmul deposits its two logits rows at the right partitions
    of a (128, 512) PSUM accumulator; 64 pairs accumulate into a full
    128-row logits tile. float32r gives the 1 cycle/row PE path.
  - softmax fused on the PSUM accumulators (DVE max, ACT exp with -max bias
    and accum_out row sums, DVE reciprocal + scale), then DMA out.
"""

import numpy as np

import concourse.bass as bass
import concourse.bacc as bacc
import concourse.mybir as mybir
from concourse import tile
from concourse.bass_utils import run_bass_kernel_spmd

K, D, E = 2048, 128, 64
NCORES = 8
R = K // NCORES            # 256 rows per core
NPAIR = 64                 # row pairs per 128-row i-tile
NT = 4                     # 512-wide j tiles
F32 = mybir.dt.float32
F32R = mybir.dt.float32r
F16 = mybir.dt.float16
AF = mybir.ActivationFunctionType
AX = mybir.AxisListType


def build_nc() -> bass.Bass:
    # Bacc (not raw Bass): its finalize() runs generate_event_semaphores(),
    # which splits multi-sem waits — TRN2 instructions hold at most one wait.
    nc = bacc.Bacc(None, target_bir_lowering=False)
    # zT/zcT come in fp16 and pre-transposed (host layout prep): plain
    # contiguous DMAs, d already on partitions for the hi/hj contractions,
    # and fp16 matmuls run 1 cyc/row.
    zT_d = nc.declare_dram_parameter("zT", [D, K], F16, isOutput=False)
    zcT_d = nc.declare_dram_parameter("zcT", [D, R], F16, isOutput=False)
    # w1a2/w1b2 = [W1a | W1a], [W1b | W1b]: one matmul emits both
    # partition-halves of the (s,e)-duplicated layouts directly.
    w1a2 = nc.declare_dram_parameter("w1a2", [D, 128], F16, isOutput=False)
    w1b2 = nc.declare_dram_parameter("w1b2", [D, 128], F16, isOutput=False)
    b1c2 = nc.declare_dram_parameter("b1c2", [128, 1], F32, isOutput=False)
    stat = nc.declare_dram_parameter("stat", [128, NPAIR, 128], F16, isOutput=False)
    out = nc.declare_dram_parameter("out", [R, K], F32, isOutput=True)

    with tile.TileContext(nc) as tc:
        with tc.tile_pool(name="singles", bufs=1) as singles:
            w1a_sb = singles.tile([D, 128], F16)
            w1b_sb = singles.tile([D, 128], F16)
            b1_sb = singles.tile([128, 1], F32)
            stat_sb = singles.tile([128, NPAIR, 128], F16)
            zT = singles.tile([128, K], F16)
            zcT = singles.tile([128, R], F16)
            hjbT2 = singles.tile([128, K], F32)
            hibP = singles.tile([128, 2 * NPAIR], F32)

            # plain contiguous loads; zT first (it gates the hjbT2 chain),
            # stat (2 MB) last — needed ~15us in.
            nc.sync.dma_start(out=zT[:], in_=zT_d[:])
            nc.sync.dma_start(out=zcT[:], in_=zcT_d[:])
            nc.sync.dma_start(out=w1a_sb[:], in_=w1a2[:])
            nc.sync.dma_start(out=w1b_sb[:], in_=w1b2[:])
            nc.sync.dma_start(out=b1_sb[:], in_=b1c2[:])
            nc.sync.dma_start(out=stat_sb[:], in_=stat[:])

            # ---- prologue: hi / hj projections ----
            with tc.tile_pool(name="pp", bufs=1, space="PSUM") as pp:
                # hiT (both halves) -> pair-bias columns; lane-aligned copies
                # (even columns land on the s=0 half, odd on s=1).
                ph = pp.tile([128, R], F32, tag="ph")
                nc.tensor.matmul(ph[:], w1a_sb[:], zcT[:], start=True, stop=True)
                phr = ph.rearrange("e (k two) -> e two k", two=2)
                nc.vector.tensor_copy(hibP[0:E, :], phr[0:E, 0, :])
                nc.vector.tensor_copy(hibP[E:128, :], phr[E:128, 1, :])

                for t in range(NT):
                    # hjT + b1, both (s,e) halves at once via [W1b|W1b].
                    pj = pp.tile([128, 512], F32, tag="pj", bufs=2)
                    nc.tensor.matmul(
                        pj[:], w1b_sb[:], zT[:, t * 512 : (t + 1) * 512],
                        start=True, stop=True,
                    )
                    nc.vector.tensor_scalar_add(
                        out=hjbT2[:, t * 512 : (t + 1) * 512],
                        in0=pj[:], scalar1=b1_sb[:],
                    )

            # ---- main loop: silu + e-contraction into PSUM accumulators ----
            with (
                tc.tile_pool(name="accp", bufs=1, space="PSUM") as accp,
                tc.tile_pool(name="hp", bufs=8) as hp,
                tc.tile_pool(name="ep", bufs=1) as ep,
                tc.tile_pool(name="sp", bufs=4) as sp,
            ):
                acc = {
                    (u, t): accp.tile(
                        [128, 512], F32, tag=f"a{u}{t}", name=f"acc{u}{t}"
                    )
                    for u in range(R // 128)
                    for t in range(NT)
                }
                def contract(k, h_ap):
                    """4 matmuls: acc rows 2kk,2kk+1 += W2-block @ silu tile"""
                    u, kk = divmod(k, NPAIR)
                    st = stat_sb[:, kk, :]
                    for t in range(NT):
                        nc.tensor.matmul(
                            acc[(u, t)][:],
                            st,
                            h_ap[:, t * 512 : (t + 1) * 512],
                            start=(kk == 0),
                            stop=(kk == NPAIR - 1),
                        )

                # Warm-up pairs on the per-pair path (no DVE dependency, so
                # silu starts the moment hjbT2/hibP are ready; also covers
                # the window where the stat DMA is still landing).
                WARM = 8
                for k in range(WARM):
                    h = hp.tile([128, K], F16, tag="h")
                    nc.scalar.activation(
                        out=h[:], in_=hjbT2[:], func=AF.Silu,
                        bias=hibP[:, k : k + 1], scale=1.0,
                    )
                    contract(k, h)

                # Steady state: DVE precomputes x = hjbT2 + bias for 4 pairs
                # (2x_2P mode), then ONE 8192-wide ScalarE silu covers all 4 —
                # amortizes the per-instruction SBUF-latency bubble.
                G = 4
                TAIL = 4  # last pairs go per-pair so the final MM+softmax
                # chain after the last silu is short
                for k0 in range(WARM, R // 2 - TAIL, G):
                    xg = hp.tile([128, G, K], F16, tag="xg", bufs=2)
                    hg = hp.tile([128, G, K], F16, tag="hg", bufs=2)
                    for g in range(G):
                        nc.vector.tensor_scalar_add(
                            out=xg[:, g, :], in0=hjbT2[:],
                            scalar1=hibP[:, k0 + g : k0 + g + 1],
                        )
                    nc.scalar.activation(
                        out=hg.rearrange("p g j -> p (g j)"),
                        in_=xg.rearrange("p g j -> p (g j)"),
                        func=AF.Silu,
                    )
                    for g in range(G):
                        contract(k0 + g, hg[:, g, :])

                for k in range(R // 2 - TAIL, R // 2):
                    h = hp.tile([128, K], F16, tag="h")
                    nc.scalar.activation(
                        out=h[:], in_=hjbT2[:], func=AF.Silu,
                        bias=hibP[:, k : k + 1], scale=1.0,
                    )
                    contract(k, h)

                # ---- fused row softmax + store ----
                # logits are O(+-6) here, so exp without max-subtraction is
                # safe in fp32 and drops the serial max chain from the tail.
                for u in range(R // 128):
                    sums = sp.tile([128, NT], F32, tag="sums")
                    tot = sp.tile([128, 1], F32, tag="tot")
                    rec = sp.tile([128, 1], F32, tag="rec")
                    ex = ep.tile([128, K], F32, tag=f"ex{u}")
                    for t in range(NT):
                        nc.scalar.activation(
                            out=ex[:, t * 512 : (t + 1) * 512],
                            in_=acc[(u, t)][:], func=AF.Exp,
                            accum_out=sums[:, t : t + 1],
                        )
                    nc.vector.reduce_sum(out=tot[:], in_=sums[:], axis=AX.X)
                    nc.vector.reciprocal(out=rec[:], in_=tot[:])
                    # chunked normalize+store so the DMA overlaps the scale
                    for c in range(2):
                        sl = slice(c * (K // 2), (c + 1) * (K // 2))
                        nc.vector.tensor_scalar_mul(
                            out=ex[:, sl], in0=ex[:, sl], scalar1=rec[:]
                        )
                        nc.sync.dma_start(
                            out=out[u * 128 : (u + 1) * 128, sl], in_=ex[:, sl]
                        )
    nc.finalize()  # Bacc.compile(): wait splitting, reg alloc, act tables
    return nc


_CACHE: dict = {}


def _get_nc() -> bass.Bass:
    if "nc" not in _CACHE:
        _CACHE["nc"] = build_nc()
    return _CACHE["nc"]


def make_in_maps(z, W1, b1, W2):
    z = np.ascontiguousarray(np.asarray(z, np.float32))
    W1 = np.asarray(W1, np.float32)
    b1 = np.asarray(b1, np.float32)
    W2 = np.asarray(W2, np.float32)

    stat = np.zeros((128, NPAIR, 128), np.float32)
    w2col = W2[:, 0]
    for kk in range(NPAIR):
        for s in range(2):
            stat[s * E : (s + 1) * E, kk, 2 * kk + s] = w2col
    stat = stat.astype(np.float16)
    b1c2 = np.ascontiguousarray(np.tile(b1, 2).reshape(128, 1))
    w1a2 = np.ascontiguousarray(np.tile(W1[:D], (1, 2)).astype(np.float16))
    w1b2 = np.ascontiguousarray(np.tile(W1[D:], (1, 2)).astype(np.float16))
    zT16 = np.ascontiguousarray(z.astype(np.float16).T)  # (D, K)

    in_maps = []
    for c in range(NCORES):
        in_maps.append(
            {
                "zT": zT16,
                "zcT": np.ascontiguousarray(zT16[:, c * R : (c + 1) * R]),
                "w1a2": w1a2,
                "w1b2": w1b2,
                "b1c2": b1c2,
                "stat": stat,
            }
        )
    return in_maps


def run(inputs: dict, trace: bool = False):
    """Run the bass kernel; returns (full_output, BassKernelResults)."""
    nc = _get_nc()
    in_maps = make_in_maps(inputs["z"], inputs["W1"], inputs["b1"], inputs["W2"])
    res = run_bass_kernel_spmd(nc, in_maps, list(range(NCORES)), trace=trace)
    full = np.concatenate([res.results[c]["out"] for c in range(NCORES)], axis=0)
    return full, res


def kernel(**inputs) -> np.ndarray:
    full, _ = run(inputs, trace=False)
    return full


# revision 54
# speedup vs baseline: 1.0143x; 1.0062x over previous
"""Trainium2 Bass kernel for nn_DeterministicAdjacency (gnn_message_passing).

Math (reference):
    hi = z @ W1[:D]            # (K, E)
    hj = z @ W1[D:]            # (K, E)
    h  = silu(hi[:,None,:] + hj[None,:,:] + b1)    # (K, K, E)
    logits = einsum('ije,eo->ij', h, W2) + b2      # (K, K)
    out = softmax(logits, axis=-1)

b2 is dropped: softmax is invariant to a constant shift.

Sharding: rows (i / query dim) split across 8 cores, 256 rows each. Each core
computes its 256 rows of logits against the full z and does local row softmax.

Per-core layout ("layout A", e on partitions):
  - hjbT2 (128p=(s,e), 2048f=j): hj^T + b1, duplicated on both partition
    halves (s = row-parity slot). Computed once, reused for every row pair.
  - hibP (128p=(s,e), 128f=k): bias columns; column k holds
    [hi[2k,:] ; hi[2k+1,:]] so one ScalarE activation instruction computes
    silu for TWO query rows x all 2048 keys x all 64 features:
        h_k[(s,e), j] = Silu(hjbT2[(s,e), j] + hibP[(s,e), k])
    128 activation instructions total = the ACT roofline for this problem.
  - contraction over e via TensorE: stationary stat_kk (128x128) holds W2
    block-diagonally (stat[(s,e), i_loc] = W2[e] iff i_loc == 2*kk+s), so
    each pair's matmul deposits its two logits rows at the right partitions
    of a (128, 512) PSUM accumulator; 64 pairs accumulate into a full
    128-row logits tile. h/stat are fp16 (1 cycle/row PE path, psum fp32).
  - steady state: DVE precomputes x = hjbT2 + bias for groups of 4 pairs so
    one 8192-wide ScalarE silu amortizes the per-instruction SBUF bubble.
  - softmax fused on the PSUM accumulators (ACT exp + accum_out row sums;
    logits are O(+-6) so max-subtraction is skipped), DVE reciprocal +
    scale, then DMA out.
"""

import numpy as np

import concourse.bass as bass
import concourse.bacc as bacc
import concourse.mybir as mybir
from concourse import tile
from concourse.bass_utils import run_bass_kernel_spmd

K, D, E = 2048, 128, 64
NCORES = 8
R = K // NCORES            # 256 rows per core
NPAIR = 64                 # row pairs per 128-row i-tile
NT = 4                     # 512-wide j tiles
F32 = mybir.dt.float32
F32R = mybir.dt.float32r
F16 = mybir.dt.float16
AF = mybir.ActivationFunctionType
AX = mybir.AxisListType


def build_nc() -> bass.Bass:
    # Bacc (not raw Bass): its finalize() runs generate_event_semaphores(),
    # which splits multi-sem waits — TRN2 instructions hold at most one wait.
    nc = bacc.Bacc(None, target_bir_lowering=False)
    # zT/zcT come in fp16 and pre-transposed (host layout prep): plain
    # contiguous DMAs, d already on partitions for the hi/hj contractions,
    # and fp16 matmuls run 1 cyc/row.
    zT_d = nc.declare_dram_parameter("zT", [D, K], F16, isOutput=False)
    zcT_d = nc.declare_dram_parameter("zcT", [D, R], F16, isOutput=False)
    # w1a2/w1b2 = [W1a | W1a], [W1b | W1b]: one matmul emits both
    # partition-halves of the (s,e)-duplicated layouts directly.
    w1a2 = nc.declare_dram_parameter("w1a2", [D, 128], F16, isOutput=False)
    w1b2 = nc.declare_dram_parameter("w1b2", [D, 128], F16, isOutput=False)
    b1c2 = nc.declare_dram_parameter("b1c2", [128, 1], F32, isOutput=False)
    stat = nc.declare_dram_parameter("stat", [128, NPAIR, 128], F16, isOutput=False)
    out = nc.declare_dram_parameter("out", [R, K], F32, isOutput=True)

    with tile.TileContext(nc) as tc:
        with tc.tile_pool(name="singles", bufs=1) as singles:
            w1a_sb = singles.tile([D, 128], F16)
            w1b_sb = singles.tile([D, 128], F16)
            b1_sb = singles.tile([128, 1], F32)
            stat_sb = singles.tile([128, NPAIR, 128], F16)
            zT = singles.tile([128, K], F16)
            zcT = singles.tile([128, R], F16)
            hjbT2 = singles.tile([128, K], F32)
            hibP = singles.tile([128, 2 * NPAIR], F32)

            # plain contiguous loads; zT first (it gates the hjbT2 chain),
            # stat (2 MB) last — needed ~15us in.
            nc.sync.dma_start(out=zT[:], in_=zT_d[:])
            nc.sync.dma_start(out=zcT[:], in_=zcT_d[:])
            nc.sync.dma_start(out=w1a_sb[:], in_=w1a2[:])
            nc.sync.dma_start(out=w1b_sb[:], in_=w1b2[:])
            nc.sync.dma_start(out=b1_sb[:], in_=b1c2[:])
            nc.sync.dma_start(out=stat_sb[:], in_=stat[:])

            # ---- prologue: hi / hj projections ----
            with tc.tile_pool(name="pp", bufs=1, space="PSUM") as pp:
                # hiT (both halves) -> pair-bias columns; lane-aligned copies
                # (even columns land on the s=0 half, odd on s=1).
                ph = pp.tile([128, R], F32, tag="ph")
                nc.tensor.matmul(ph[:], w1a_sb[:], zcT[:], start=True, stop=True)
                phr = ph.rearrange("e (k two) -> e two k", two=2)
                nc.vector.tensor_copy(hibP[0:E, :], phr[0:E, 0, :])
                nc.vector.tensor_copy(hibP[E:128, :], phr[E:128, 1, :])

                for t in range(NT):
                    # hjT + b1, both (s,e) halves at once via [W1b|W1b].
                    pj = pp.tile([128, 512], F32, tag="pj", bufs=2)
                    nc.tensor.matmul(
                        pj[:], w1b_sb[:], zT[:, t * 512 : (t + 1) * 512],
                        start=True, stop=True,
                    )
                    nc.vector.tensor_scalar_add(
                        out=hjbT2[:, t * 512 : (t + 1) * 512],
                        in0=pj[:], scalar1=b1_sb[:],
                    )

            # ---- main loop: silu + e-contraction into PSUM accumulators ----
            with (
                tc.tile_pool(name="accp", bufs=1, space="PSUM") as accp,
                tc.tile_pool(name="hp", bufs=8) as hp,
                tc.tile_pool(name="ep", bufs=1) as ep,
                tc.tile_pool(name="sp", bufs=4) as sp,
            ):
                # one 4-bank psum tile per i-tile: matmuls write bank slices,
                # the softmax exp reads all 2048 columns in one instruction
                acc = {
                    u: accp.tile([128, NT, 512], F32, tag=f"a{u}", name=f"acc{u}")
                    for u in range(R // 128)
                }
                def contract(k, h_ap):
                    """4 matmuls: acc rows 2kk,2kk+1 += W2-block @ silu tile"""
                    u, kk = divmod(k, NPAIR)
                    st = stat_sb[:, kk, :]
                    for t in range(NT):
                        nc.tensor.matmul(
                            acc[u][:, t, :],
                            st,
                            h_ap[:, t * 512 : (t + 1) * 512],
                            start=(kk == 0),
                            stop=(kk == NPAIR - 1),
                        )

                # Warm-up pairs on the per-pair path (no DVE dependency, so
                # silu starts the moment hjbT2/hibP are ready; also covers
                # the window where the stat DMA is still landing).
                WARM = 8
                for k in range(WARM):
                    h = hp.tile([128, K], F16, tag="h")
                    nc.scalar.activation(
                        out=h[:], in_=hjbT2[:], func=AF.Silu,
                        bias=hibP[:, k : k + 1], scale=1.0,
                    )
                    contract(k, h)

                # Steady state: DVE precomputes x = hjbT2 + bias for 4 pairs
                # (2x_2P mode), then ONE 8192-wide ScalarE silu covers all 4 —
                # amortizes the per-instruction SBUF-latency bubble.
                G = 4
                TAIL = 4  # last pairs go per-pair so the final MM+softmax
                # chain after the last silu is short
                for k0 in range(WARM, R // 2 - TAIL, G):
                    xg = hp.tile([128, G, K], F16, tag="xg", bufs=2)
                    hg = hp.tile([128, G, K], F16, tag="hg", bufs=2)
                    for g in range(G):
                        nc.vector.tensor_scalar_add(
                            out=xg[:, g, :], in0=hjbT2[:],
                            scalar1=hibP[:, k0 + g : k0 + g + 1],
                        )
                    nc.scalar.activation(
                        out=hg.rearrange("p g j -> p (g j)"),
                        in_=xg.rearrange("p g j -> p (g j)"),
                        func=AF.Silu,
                    )
                    for g in range(G):
                        contract(k0 + g, hg[:, g, :])

                for k in range(R // 2 - TAIL, R // 2):
                    h = hp.tile([128, K], F16, tag="h")
                    nc.scalar.activation(
                        out=h[:], in_=hjbT2[:], func=AF.Silu,
                        bias=hibP[:, k : k + 1], scale=1.0,
                    )
                    contract(k, h)

                # ---- fused row softmax + store ----
                # logits are O(+-6) here, so exp without max-subtraction is
                # safe in fp32 and drops the serial max chain from the tail.
                for u in range(R // 128):
                    tot = sp.tile([128, 1], F32, tag="tot")
                    rec = sp.tile([128, 1], F32, tag="rec")
                    ex = ep.tile([128, K], F32, tag=f"ex{u}")
                    nc.scalar.activation(
                        out=ex.rearrange("p (t j) -> p t j", t=NT),
                        in_=acc[u][:], func=AF.Exp,
                        accum_out=tot[:],
                    )
                    nc.vector.reciprocal(out=rec[:], in_=tot[:])
                    # chunked normalize+store so the DMA overlaps the scale
                    for c in range(2):
                        sl = slice(c * (K // 2), (c + 1) * (K // 2))
                        nc.vector.tensor_scalar_mul(
                            out=ex[:, sl], in0=ex[:, sl], scalar1=rec[:]
                        )
                        nc.sync.dma_start(
                            out=out[u * 128 : (u + 1) * 128, sl], in_=ex[:, sl]
                        )
    nc.finalize()  # Bacc.compile(): wait splitting, reg alloc, act tables
    return nc


_CACHE: dict = {}


def _get_nc() -> bass.Bass:
    if "nc" not in _CACHE:
        _CACHE["nc"] = build_nc()
    return _CACHE["nc"]


def make_in_maps(z, W1, b1, W2):
    z = np.ascontiguousarray(np.asarray(z, np.float32))
    W1 = np.asarray(W1, np.float32)
    b1 = np.asarray(b1, np.float32)
    W2 = np.asarray(W2, np.float32)

    stat = np.zeros((128, NPAIR, 128), np.float32)
    w2col = W2[:, 0]
    for kk in range(NPAIR):
        for s in range(2):
            stat[s * E : (s + 1) * E, kk, 2 * kk + s] = w2col
    stat = stat.astype(np.float16)
    b1c2 = np.ascontiguousarray(np.tile(b1, 2).reshape(128, 1))
    w1a2 = np.ascontiguousarray(np.tile(W1[:D], (1, 2)).astype(np.float16))
    w1b2 = np.ascontiguousarray(np.tile(W1[D:], (1, 2)).astype(np.float16))
    zT16 = np.ascontiguousarray(z.astype(np.float16).T)  # (D, K)

    in_maps = []
    for c in range(NCORES):
        in_maps.append(
            {
                "zT": zT16,
                "zcT": np.ascontiguousarray(zT16[:, c * R : (c + 1) * R]),
                "w1a2": w1a2,
                "w1b2": w1b2,
                "b1c2": b1c2,
                "stat": stat,
            }
        )
    return in_maps


def run(inputs: dict, trace: bool = False):
    """Run the bass kernel; returns (full_output, BassKernelResults)."""
    nc = _get_nc()
    in_maps = make_in_maps(inputs["z"], inputs["W1"], inputs["b1"], inputs["W2"])
    res = run_bass_kernel_spmd(nc, in_maps, list(range(NCORES)), trace=trace)
    full = np.concatenate([res.results[c]["out"] for c in range(NCORES)], axis=0)
    return full, res


def kernel(**inputs) -> np.ndarray:
    full, _ = run(inputs, trace=False)
    return full
